# revision 5
# baseline (speedup 1.0000x reference)
"""Trainium2 Bass kernel for nn_Attn_52432960749709.

Computes, for E:[B,N,D], W1/W2:[D,D]:
    q = E @ W1 ; k = E @ W2
    scores = (q @ k^T) / sqrt(D)          # per batch, [N, N]
    out = softmax(scores, axis=1)         # normalize over rows n, per column m

Strategy (data parallel over B across 8 NeuronCores, one batch element per
core; the small DxD weights are folded on the host into M = W1 @ W2^T and
replicated):

    scores = E M E^T / sqrt(D)
    Per core:
      E^T   via PE transposes (fp32)
      G^T   = M E^T            (f32r matmuls)
      s^T   = (E) (G^T)        tiles [128 m, 512 n]  (f32r matmuls)
      softmax along the free (n) axis:
        pass1: ACT exp(scale*s) -> SBUF strip, accum_out gives Z partials
        DVE:   Z = sum partials ; R = 1/Z
        pass2: ACT copy*R -> fp16 strip
      output transpose-back via PE transposes (fp16), DMA to HBM
    Host upcasts the fp16 output to fp32.
"""

import math

import numpy as np

B, N, D = 8, 2048, 512
P = 128
DC = D // P  # 4 contraction chunks
NB = 512  # matmul moving free dim
NBS = N // NB  # 4 n-blocks per row strip
MC = N // P  # 16 m-chunks per core

_CACHE: dict = {}


def _patch_tile_drain():
    """This walrus build rejects >1 extra sem wait on one TPB_CTRL
    instruction, so split the end-of-kernel drain's wait set across chained
    SP NOPs (same engine, so program order preserves barrier semantics)."""
    import concourse.tile as tile
    from concourse.vector_clock import ScopedClock

    if getattr(tile.TileContext, "_drain_split_patched", False):
        return

    max_waits = 1

    def _drain_and_barrier_split(self, tick_clock, wait_clock):
        nc = self.nc
        drain_inst = nc.sync.drain()
        wait_clock.add_sem_waits(
            drain_inst.ins, ScopedClock({None: tick_clock.global_clock})
        )
        si = drain_inst.ins.sync_info
        waits = list(si.on_wait or []) if si is not None else []
        if len(waits) > max_waits:
            si.on_wait = waits[:max_waits]
            rest = waits[max_waits:]
            while rest:
                nop = nc.sync.nop(nofuse=True, hint="drain_wait_split")
                chunk, rest = rest[:max_waits], rest[max_waits:]
                nsi = nop.ins.sync_info
                if nsi is None:
                    import bass_rust

                    nop.ins.sync_info = bass_rust.SyncInfo(
                        on_wait=chunk, on_update=[]
                    )
                else:
                    nsi.on_wait = chunk

        nc.all_engine_barrier()
        assert self.sems is not None
        popped = nc._tile_sem_poison_stack.pop()
        assert popped is self._sem_poison
        nc.clear_and_free_semaphores(list(self.sems.allocated().values()))
        nc.all_engine_barrier()

    tile.TileContext._drain_and_barrier = _drain_and_barrier_split
    tile.TileContext._drain_split_patched = True


def _split_multi_waits(nc):
    """This walrus build supports only one sem-wait command per instruction.
    Hoist extra waits onto same-engine NOPs inserted just before the
    instruction (engines execute in order, so semantics are preserved)."""
    import bass_rust
    import concourse.mybir as mybir

    ctr = 0
    for fn in nc.m.functions:
        for blk in fn.blocks:
            insts = blk.instructions
            out = []
            changed = False
            for inst in insts:
                si = inst.sync_info
                waits = list(si.on_wait) if (si is not None and si.on_wait) else []
                if len(waits) > 1:
                    changed = True
                    for w in waits[:-1]:
                        ctr += 1
                        nop = mybir.InstNoOp(name=f"I-waitsplit-{ctr}")
                        nop.engine = inst.engine
                        nop.sync_info = bass_rust.SyncInfo(
                            on_wait=[w], on_update=[]
                        )
                        out.append(nop)
                    si.on_wait = waits[-1:]
                out.append(inst)
            if changed:
                blk.instructions = out


def _build_nc():
    import concourse.bass as bass
    import concourse.mybir as mybir
    import concourse.tile as tile
    from concourse.masks import make_identity

    _patch_tile_drain()

    dt = mybir.dt
    f32, f32r, f16 = dt.float32, dt.float32r, dt.float16
    Exp = mybir.ActivationFunctionType.Exp
    X = mybir.AxisListType.X

    scale = 1.0 / math.sqrt(float(D))

    nc = bass.Bass()
    E_d = nc.dram_tensor("E", [N, D], f32, kind="ExternalInput")
    M_d = nc.dram_tensor("M", [D, D], f32, kind="ExternalInput")
    O_d = nc.dram_tensor("O", [N, N], f16, kind="ExternalOutput")
    # Output viewed as [p, j, m] with n = j*128 + p for the transpose-back DMA.
    O_r = O_d[:].rearrange("(j p) m -> p j m", p=P)

    with tile.TileContext(nc) as tc:
        with (
            tc.tile_pool(name="persist", bufs=1) as persist,
            tc.tile_pool(name="ein", bufs=4) as ein,
            tc.tile_pool(name="exps", bufs=3) as exps,
            tc.tile_pool(name="outs", bufs=3) as outs,
            tc.tile_pool(name="ots", bufs=3) as otsp,
            tc.tile_pool(name="small", bufs=8) as small,
            tc.tile_pool(name="psum_s", bufs=5, space="PSUM") as psum_s,
            tc.tile_pool(name="psum_t", bufs=3, space="PSUM") as psum_t,
        ):
            ident32 = persist.tile([P, P], f32, tag="id32")
            make_identity(nc, ident32)
            ident16 = persist.tile([P, P], f16, tag="id16")
            make_identity(nc, ident16)

            # f32r: fp32 storage pre-rounded for single-pass PE matmul; the
            # producing copy instruction performs the rounding.
            ET = persist.tile([P, DC, N], f32r, tag="ET")  # E^T  [d, n]
            GT = persist.tile([P, DC, N], f32r, tag="GT")  # G^T  [d', n]
            Mraw = persist.tile([P, DC, D], f32, tag="Mraw")
            Msb = persist.tile([P, DC, D], f32r, tag="M")  # M    [d, d']

            for c in range(DC):
                nc.sync.dma_start(Mraw[:, c, :], M_d[c * P : (c + 1) * P, :])
            nc.vector.tensor_copy(out=Msb[:], in_=Mraw[:])

            # ---- E^T via PE transposes ----
            for i in range(MC):
                et = ein.tile([P, D], f32, tag="et")
                nc.sync.dma_start(et, E_d[i * P : (i + 1) * P, :])
                ps = psum_s.tile([P, NB], f32, tag="ps")
                for j in range(DC):
                    nc.tensor.transpose(
                        ps[:, j * P : (j + 1) * P], et[:, j * P : (j + 1) * P], ident32
                    )
                nc.vector.tensor_copy(
                    out=ET[:, :, i * P : (i + 1) * P],
                    in_=ps.rearrange("p (c n) -> p c n", c=DC),
                )

            # ---- G^T = M E^T ----
            for dpc in range(DC):
                for nb in range(NBS):
                    ps = psum_s.tile([P, NB], f32, tag="ps")
                    for dc in range(DC):
                        nc.tensor.matmul(
                            ps,
                            lhsT=Msb[:, dc, dpc * P : (dpc + 1) * P],
                            rhs=ET[:, dc, nb * NB : (nb + 1) * NB],
                            start=(dc == 0),
                            stop=(dc == DC - 1),
                        )
                    nc.vector.tensor_copy(
                        out=GT[:, dpc, nb * NB : (nb + 1) * NB], in_=ps
                    )

            # ---- main loop over m-chunks, software-pipelined ----
            # stage a(mi): scores matmuls + exp pass1
            # stage b(mi): 1/Z + normalize pass2       (emitted at iter mi+1)
            # stage c(mi): transpose-back + copies + DMA (emitted at iter mi+2)
            ssbs: dict = {}
            osbs: dict = {}

            def stage_a(mi):
                ssb = exps.tile([P, N], f32, tag="ssb")
                zp = small.tile([P, NBS], f32, tag="zp")
                for nb in range(NBS):
                    ps = psum_s.tile([P, NB], f32, tag="ps")
                    for dc in range(DC):
                        nc.tensor.matmul(
                            ps,
                            lhsT=ET[:, dc, mi * P : (mi + 1) * P],
                            rhs=GT[:, dc, nb * NB : (nb + 1) * NB],
                            start=(dc == 0),
                            stop=(dc == DC - 1),
                        )
                    nc.scalar.activation(
                        ssb[:, nb * NB : (nb + 1) * NB],
                        ps,
                        Exp,
                        scale=scale,
                        accum_out=zp[:, nb : nb + 1],
                    )
                # normalizer on DVE right away (cheap, keeps ACT unblocked)
                zs = small.tile([P, 1], f32, tag="zs")
                nc.vector.reduce_sum(zs, zp, axis=X)
                rv = small.tile([P, 1], f32, tag="rv")
                nc.vector.reciprocal(rv, zs)
                ssbs[mi] = (ssb, rv)

            def stage_b(mi):
                ssb, rv = ssbs.pop(mi)
                osb = outs.tile([P, N], f16, tag="osb")
                for nb in range(NBS):
                    nc.scalar.mul(
                        osb[:, nb * NB : (nb + 1) * NB],
                        ssb[:, nb * NB : (nb + 1) * NB],
                        rv,
                    )
                osbs[mi] = osb

            def stage_c(mi):
                osb = osbs.pop(mi)
                ots = otsp.tile([P, MC, P], f16, tag="ots")
                for j in range(MC):
                    pst = psum_t.tile([P, P], f16, tag="pst")
                    nc.tensor.transpose(
                        pst, osb[:, j * P : (j + 1) * P], ident16
                    )
                    nc.vector.tensor_copy(out=ots[:, j, :], in_=pst)
                nc.sync.dma_start(O_r[:, :, mi * P : (mi + 1) * P], ots)

            for mi in range(MC):
                stage_a(mi)
                if mi >= 1:
                    stage_b(mi - 1)
                if mi >= 2:
                    stage_c(mi - 2)
            stage_b(MC - 1)
            stage_c(MC - 2)
            stage_c(MC - 1)

    _split_multi_waits(nc)
    return nc


def _get_runner():
    """Build (once) the jitted 8-core SPMD executable.

    Returns (fn, pack) where pack(in_maps) -> list of device-ready args and
    fn(*args) -> tuple of concatenated outputs ([8*N, N] fp16)."""
    if "runner" in _CACHE:
        return _CACHE["runner"]

    import jax
    import numpy as _np
    from jax.sharding import Mesh, PartitionSpec
    from jax.experimental.shard_map import shard_map

    import concourse.mybir as mybir
    from concourse import bass2jax

    nc = _build_nc()
    bass2jax.install_neuronx_cc_hook()

    partition_name = (
        nc.partition_id_tensor.name if nc.partition_id_tensor else None
    )

    in_names = []
    out_names = []
    out_avals = []
    for alloc in nc.m.functions[0].allocations:
        if not isinstance(alloc, mybir.MemoryLocationSet):
            continue
        name = alloc.memorylocations[0].name
        if alloc.kind == "ExternalInput":
            if name != partition_name:
                in_names.append(name)
        elif alloc.kind == "ExternalOutput":
            out_names.append(name)
            out_avals.append(
                jax.core.ShapedArray(
                    tuple(alloc.tensor_shape), mybir.dt.np(alloc.dtype)
                )
            )
    n_params = len(in_names)
    n_outs = len(out_avals)
    in_names_all = list(in_names) + list(out_names)
    if partition_name is not None:
        in_names_all.append(partition_name)

    def _body(*args):
        operands = list(args)
        if partition_name is not None:
            operands.append(bass2jax.partition_id_tensor())
        outs = bass2jax._bass_exec_p.bind(
            *operands,
            out_avals=tuple(out_avals),
            in_names=tuple(in_names_all),
            out_names=tuple(out_names),
            lowering_input_output_aliases=(),
            sim_require_finite=True,
            sim_require_nnan=True,
            nc=nc,
        )
        return tuple(outs)

    devices = jax.devices()[:B]
    mesh = Mesh(_np.asarray(devices), ("core",))
    in_specs = (PartitionSpec("core"),) * (n_params + n_outs)
    out_specs = (PartitionSpec("core"),) * n_outs
    fn = jax.jit(
        shard_map(
            _body, mesh=mesh, in_specs=in_specs, out_specs=out_specs, check_rep=False
        ),
        keep_unused=True,
    )

    def pack(in_maps):
        concat_in = [
            _np.concatenate([_np.asarray(m[name]) for m in in_maps], axis=0)
            for name in in_names[:n_params]
        ]
        concat_zero = [
            _np.zeros((B * a.shape[0], *a.shape[1:]), a.dtype) for a in out_avals
        ]
        return [jax.device_put(a) for a in concat_in + concat_zero]

    _CACHE["runner"] = (fn, pack, out_names, out_avals)
    return _CACHE["runner"]


def kernel(E, W1, W2):
    E = np.ascontiguousarray(np.asarray(E), dtype=np.float32)
    W1 = np.asarray(W1, dtype=np.float32)
    W2 = np.asarray(W2, dtype=np.float32)
    # Fold the two projections: scores = E (W1 W2^T) E^T. Done in float64 on
    # host for accuracy; negligible cost (512^3 FLOPs).
    Mw = (W1.astype(np.float64) @ W2.astype(np.float64).T).astype(np.float32)

    fn, pack, out_names, out_avals = _get_runner()
    in_maps = [{"E": E[b], "M": Mw} for b in range(B)]
    args = pack(in_maps)
    outs = fn(*args)
    o = np.asarray(outs[0])  # [8*N, N] fp16
    return o.reshape(B, N, N).astype(np.float32)


if __name__ == "__main__":
    rng = np.random.default_rng(0)
    E = rng.standard_normal((B, N, D), dtype=np.float32)
    W1 = rng.standard_normal((D, D), dtype=np.float32) * (2.0 / (D + D)) ** 0.5
    W2 = rng.standard_normal((D, D), dtype=np.float32) * (2.0 / (D + D)) ** 0.5
    out = kernel(E=E, W1=W1, W2=W2)
    print(out.shape, out.dtype, out.sum())
